# revision 3
# baseline (speedup 1.0000x reference)
"""Causal multi-head self-attention (B=2, L=2048, D=1024, h=16, RoPE) on 8 TRN2
NeuronCores, tensor-parallel over heads (2 heads/core), host-side sum of the
per-core partial W_o outputs."""

import sys

try:
    import concourse  # noqa: F401
except ImportError:
    sys.path.insert(0, "/opt/trn_rl_repo")

import numpy as np

import concourse.bass as bass
import concourse.mybir as mybir
import concourse.tile as tile
from concourse import bacc
from concourse.bass import ts
from concourse.bass_utils import run_bass_kernel_spmd

F32 = mybir.dt.float32
F32R = mybir.dt.float32r
ActF = mybir.ActivationFunctionType
Alu = mybir.AluOpType

B, L, D = 2, 2048, 1024
H, DH = 16, 64           # heads, head dim
T = B * L                # 4096 tokens
NC = 8                   # cores
HPC = H // NC            # 2 heads per core
DPC = HPC * DH           # 128 dims per core
NB = L // 512            # 4 q-blocks of 512 per batch
KB = L // 128            # 16 k-blocks of 128 per batch
THETA = 10000.0

# set by test harness: run with trace and record exec time
TRACE = False
LAST_EXEC_NS = None
LAST_RESULTS = None

_cache = {}


def _round_f32r(a: np.ndarray) -> np.ndarray:
    """Round fp32 to the PE's fp32r format (RNE on the low 12 mantissa bits)."""
    b = np.ascontiguousarray(a, dtype=np.float32).view(np.uint32).astype(np.uint64)
    r = ((b + 0x800) & 0xFFFFF000).astype(np.uint32)
    return r.view(np.float32).reshape(a.shape)


def _build_nc():
    nc = bacc.Bacc("TRN2", target_bir_lowering=False, debug=False)

    xT = nc.dram_tensor("xT", [D, T], F32R, kind="ExternalInput")
    wqT = nc.dram_tensor("wqT", [D, DPC], F32R, kind="ExternalInput")
    wkT = nc.dram_tensor("wkT", [D, DPC], F32R, kind="ExternalInput")
    wvT = nc.dram_tensor("wvT", [D, DPC], F32R, kind="ExternalInput")
    woC = nc.dram_tensor("woC", [DPC, D], F32R, kind="ExternalInput")
    cosP = nc.dram_tensor("cosP", [DPC, L], F32, kind="ExternalInput")
    sinP2 = nc.dram_tensor("sinP2", [DPC, L], F32, kind="ExternalInput")
    masks = nc.dram_tensor("masks", [128, 4, 512], F32, kind="ExternalInput")
    ident = nc.dram_tensor("ident", [128, 128], F32R, kind="ExternalInput")
    out = nc.dram_tensor("out", [T, D], F32, kind="ExternalOutput")

    with tile.TileContext(nc) as tc:
        with (
            tc.tile_pool(name="const", bufs=1) as cpool,
            tc.tile_pool(name="xp", bufs=1) as xpool,
            tc.tile_pool(name="qkv", bufs=1) as qkvpool,
            tc.tile_pool(name="vaugp", bufs=1) as vaugpool,
            tc.tile_pool(name="rope", bufs=2) as ropepool,
            tc.tile_pool(name="pexp", bufs=6) as pexppool,
            tc.tile_pool(name="attn", bufs=1) as attnpool,
            tc.tile_pool(name="small", bufs=2) as smallpool,
            tc.tile_pool(name="outp", bufs=4) as outpool,
            tc.tile_pool(name="ps_proj", bufs=2, space="PSUM") as ps_proj,
            tc.tile_pool(name="ps_tp", bufs=1, space="PSUM") as ps_tp,
            tc.tile_pool(name="ps_st", bufs=2, space="PSUM") as ps_st,
            tc.tile_pool(name="ps_pv", bufs=2, space="PSUM") as ps_pv,
            tc.tile_pool(name="ps_wo", bufs=1, space="PSUM") as ps_wo,
        ):
            # ---- persistent constants ----
            wq_t = cpool.tile([128, 8, DPC], F32R)
            wk_t = cpool.tile([128, 8, DPC], F32R)
            wv_t = cpool.tile([128, 8, DPC], F32R)
            for dst, src in ((wq_t, wqT), (wk_t, wkT), (wv_t, wvT)):
                nc.sync.dma_start(dst[:], src.rearrange("(ko ki) m -> ki ko m", ki=128))
            woC_t = cpool.tile([128, D], F32R)
            nc.sync.dma_start(woC_t[:], woC[:, :])
            cos_t = cpool.tile([128, L], F32)
            nc.sync.dma_start(cos_t[:], cosP[:, :])
            sin_t = cpool.tile([128, L], F32)
            nc.sync.dma_start(sin_t[:], sinP2[:, :])
            mask_t = cpool.tile([128, 4, 512], F32)
            nc.sync.dma_start(mask_t[:], masks[:, :, :])
            ident_t = cpool.tile([128, 128], F32R)
            nc.sync.dma_start(ident_t[:], ident[:, :])

            for b in range(B):
                # ---- load xT for this batch ----
                xT_t = xpool.tile([128, 8, L], F32R, tag="xT")
                nc.sync.dma_start(
                    xT_t[:],
                    xT.rearrange("(ko ki) t -> ki ko t", ki=128)[:, :, ts(b, L)],
                )

                # ---- projections (+ fused RoPE for Q, K) ----
                qtf = qkvpool.tile([128, L], F32R, tag="qtf")
                ktf = qkvpool.tile([128, L], F32R, tag="ktf")
                vt = qkvpool.tile([128, L], F32R, tag="vt")
                for wt, dst, is_v in ((wq_t, qtf, False), (wk_t, ktf, False), (wv_t, vt, True)):
                    for nb in range(NB):
                        ps = ps_proj.tile([128, 512], F32, tag="proj")
                        for k in range(8):
                            nc.tensor.matmul(
                                ps[:],
                                wt[:, k, :],
                                xT_t[:, k, ts(nb, 512)],
                                start=(k == 0),
                                stop=(k == 7),
                            )
                        if is_v:
                            nc.vector.tensor_copy(dst[:, ts(nb, 512)], ps[:])
                        else:
                            u = ropepool.tile([128, 512], F32, tag="u")
                            w = ropepool.tile([128, 512], F32, tag="w")
                            wsw = ropepool.tile([128, 512], F32, tag="wsw")
                            nc.vector.tensor_mul(u[:], ps[:], cos_t[:, ts(nb, 512)])
                            nc.vector.tensor_mul(w[:], ps[:], sin_t[:, ts(nb, 512)])
                            # swap the 32-row halves within each head's 64 rows
                            for blk, src_blk in enumerate((1, 0, 3, 2)):
                                nc.sync.dma_start(
                                    wsw[32 * blk : 32 * blk + 32, :],
                                    w[32 * src_blk : 32 * src_blk + 32, :],
                                )
                            nc.vector.tensor_add(dst[:, ts(nb, 512)], u[:], wsw[:])

                # ---- V_aug: natural-layout V with a ones column per head ----
                vaug = vaugpool.tile([128, KB, 130], F32R, tag="vaug")
                for kb in range(KB):
                    pst = ps_tp.tile([128, 128], F32R, tag="tp")
                    nc.tensor.transpose(pst[:], vt[:, ts(kb, 128)], ident_t[:])
                    nc.vector.tensor_copy(vaug[:, kb, 0:64], pst[:, 0:64])
                    nc.vector.tensor_copy(vaug[:, kb, 65:129], pst[:, 64:128])
                    nc.vector.tensor_scalar(
                        vaug[:, kb, 64:65], pst[:, 0:1], 0.0, 1.0, Alu.mult, Alu.add
                    )
                    nc.vector.tensor_scalar(
                        vaug[:, kb, 129:130], pst[:, 0:1], 0.0, 1.0, Alu.mult, Alu.add
                    )

                # ---- causal attention, k-partition layout ----
                attn_t = attnpool.tile([128, L], F32R, tag="attnT")
                for j in range(NB):
                    for h in range(HPC):
                        hp = 64 * h
                        pv_ps = ps_pv.tile([65, 512], F32, tag="pv")
                        nkb = 4 * j + 4
                        for kb in range(nkb):
                            st_ps = ps_st.tile([128, 512], F32, tag="st")
                            nc.tensor.matmul(
                                st_ps[:],
                                ktf[hp : hp + 64, ts(kb, 128)],
                                qtf[hp : hp + 64, ts(j, 512)],
                                start=True,
                                stop=True,
                            )
                            mi = kb - 4 * j
                            if mi >= 0:
                                nc.vector.tensor_add(st_ps[:], st_ps[:], mask_t[:, mi, :])
                            pexp = pexppool.tile([128, 512], F32R, tag="pexp")
                            nc.scalar.activation(pexp[:], st_ps[:], ActF.Exp, scale=0.125)
                            nc.tensor.matmul(
                                pv_ps[:],
                                vaug[:, kb, 65 * h : 65 * h + 65],
                                pexp[:],
                                start=(kb == 0),
                                stop=(kb == nkb - 1),
                            )
                        # softmax denominators: row 64 of pv_ps (PSUM is not
                        # DMA-readable: engine-copy at base 64, DMA-shift to
                        # partition 0, then gpsimd broadcast)
                        ltmp = smallpool.tile([65, 512], F32, tag="ltmp")
                        nc.vector.tensor_copy(ltmp[64:65, :], pv_ps[64:65, :])
                        lrow = smallpool.tile([1, 512], F32, tag="lrow")
                        nc.sync.dma_start(lrow[:], ltmp[64:65, :])
                        llh = smallpool.tile([64, 512], F32, tag="llh")
                        nc.gpsimd.partition_broadcast(llh[:], lrow[:])
                        nc.vector.reciprocal(llh[:], llh[:])
                        if h == 0:
                            nc.vector.tensor_mul(
                                attn_t[0:64, ts(j, 512)], pv_ps[0:64, :], llh[:]
                            )
                        else:
                            nrm = smallpool.tile([64, 512], F32R, tag="nrm")
                            nc.vector.tensor_mul(nrm[:], pv_ps[0:64, :], llh[:])
                            nc.sync.dma_start(attn_t[64:128, ts(j, 512)], nrm[:])

                # ---- partial W_o ----
                for qb in range(KB):
                    for nh in range(2):
                        wo_ps = ps_wo.tile([128, 512], F32, tag="wo")
                        nc.tensor.matmul(
                            wo_ps[:],
                            attn_t[:, ts(qb, 128)],
                            woC_t[:, ts(nh, 512)],
                            start=True,
                            stop=True,
                        )
                        osb = outpool.tile([128, 512], F32, tag="osb")
                        nc.vector.tensor_copy(osb[:], wo_ps[:])
                        nc.sync.dma_start(
                            out[b * L + qb * 128 : b * L + qb * 128 + 128, ts(nh, 512)],
                            osb[:],
                        )
    nc.compile()
    return nc


def _host_inputs(x, W_q, W_k, W_v, W_o, token_positions):
    """Build per-core input maps (host-side layout preprocessing only)."""
    # interleaved->rotate-half permutation of head dims, folded into W_q / W_k
    perm = np.empty(D, dtype=np.int64)
    for gh in range(H):
        base = gh * DH
        for i in range(DH // 2):
            perm[base + i] = base + 2 * i
            perm[base + DH // 2 + i] = base + 2 * i + 1
    Wq_p = np.asarray(W_q, np.float32)[perm, :]
    Wk_p = np.asarray(W_k, np.float32)[perm, :]
    Wv = np.asarray(W_v, np.float32)
    Wo = np.asarray(W_o, np.float32)

    xT = _round_f32r(np.asarray(x, np.float32).reshape(T, D).T)

    # RoPE tables (angles in f32 to match the reference's f32 computation)
    pos = np.asarray(token_positions).astype(np.float32)
    inv_freq = (THETA ** (-(np.arange(DH // 2, dtype=np.float32) * 2.0) / DH)).astype(
        np.float32
    )
    ang = (pos[:, None] * inv_freq[None, :]).astype(np.float32)  # [L, 32]
    cos = np.cos(ang.astype(np.float64)).astype(np.float32)  # [L, 32]
    sin = np.sin(ang.astype(np.float64)).astype(np.float32)
    cosP = np.empty((DPC, L), np.float32)
    sinP2 = np.empty((DPC, L), np.float32)
    for lh in range(HPC):
        r0 = 64 * lh
        cosP[r0 : r0 + 32, :] = cos.T
        cosP[r0 + 32 : r0 + 64, :] = cos.T
        sinP2[r0 : r0 + 32, :] = sin.T          # x1 rows: +sin
        sinP2[r0 + 32 : r0 + 64, :] = -sin.T    # x2 rows: -sin
    cosP = np.ascontiguousarray(cosP)
    sinP2 = np.ascontiguousarray(sinP2)

    # additive causal masks for the 4 diagonal 128x512 block offsets
    r = np.arange(128)[:, None]
    c = np.arange(512)[None, :]
    masks = np.stack(
        [np.where(r + 128 * mi <= c, 0.0, -1e30).astype(np.float32) for mi in range(4)],
        axis=1,
    )  # [128, 4, 512]
    masks = np.ascontiguousarray(masks)

    ident = _round_f32r(np.eye(128, dtype=np.float32))

    in_maps = []
    for core in range(NC):
        sl = slice(DPC * core, DPC * (core + 1))
        in_maps.append(
            {
                "xT": xT,
                "wqT": _round_f32r(Wq_p[sl, :].T),
                "wkT": _round_f32r(Wk_p[sl, :].T),
                "wvT": _round_f32r(Wv[sl, :].T),
                "woC": _round_f32r(Wo[:, sl].T),
                "cosP": cosP,
                "sinP2": sinP2,
                "masks": masks,
                "ident": ident,
            }
        )
    return in_maps


def kernel(x, W_q, W_k, W_v, W_o, token_positions):
    global LAST_EXEC_NS, LAST_RESULTS
    if "nc" not in _cache:
        _cache["nc"] = _build_nc()
    nc = _cache["nc"]
    in_maps = _host_inputs(x, W_q, W_k, W_v, W_o, token_positions)
    res = run_bass_kernel_spmd(nc, in_maps, list(range(NC)), trace=TRACE)
    LAST_EXEC_NS = res.exec_time_ns
    LAST_RESULTS = res
    total = np.zeros((T, D), dtype=np.float64)
    for core in range(NC):
        total += res.results[core]["out"].astype(np.float64)
    return total.reshape(B, L, D).astype(np.float32)
